# revision 1
# baseline (speedup 1.0000x reference)
"""Trainium2 Bass kernel for nn_MultiHeadAttention (B=2, S=2048, d_model=1024, H=16).

Sharding (8 cores): data-parallel over B (2) x tensor-parallel over head groups
(4 groups of 4 heads).  Each core computes its head-group's Q/K/V projections
(column-sharded weights), attention for its 4 heads, and a row-parallel
out_proj partial product.  The host sums the 4 partials per batch (the
"all-reduce") and adds the output bias.

All on-chip layouts are transposed ([feature, seq]) so that:
  - scores are computed directly transposed  S_T[k,q] = Kh @ Qh^T  (no P
    transpose needed before P@V),
  - softmax denominators come from ones-vector matmuls (col-tiled 4-way),
  - the PE array is fully packed for dk=64 heads via row/col tile_position
    pairing (auto-derived from AP base partitions),
  - the k-loop is software-pipelined one stage (scores of tile k overlap
    exp and P@V of tile k-1), inputs stream on both HWDGE queues.

Dtypes: inputs/projections and the P,V operands are fp16 (1 cyc/row on
the PE, fp32 PSUM accumulation everywhere); scores and out_proj operands
are float32r (TF32 path, 1 cyc/row at N>=256; note f32r cannot be
col-tiled -- XBUS budget -- which is why the P@V/sums side is fp16).
"""

import sys
import numpy as np

for _p in ("/opt/trn_rl_repo", "/root/.axon_site/_ro/trn_rl_repo"):
    if _p not in sys.path:
        sys.path.append(_p)

D_MODEL = 1024
NUM_HEADS = 16
DK = 64
B = 2
S = 2048
N_CORES = 8
HPC = 4               # heads per core
E = HPC * DK          # 256 features per core
NQ = 512              # q-chunk size
N_QC = S // NQ        # 4 q chunks
N_KT = S // 128       # 16 k tiles
N_DT = D_MODEL // 128  # 8 contraction tiles for projections

_PROGRAM = None
_RUN_KWARGS = {}      # test harness may set {"trace": True}
_LAST_RESULTS = None  # BassKernelResults of the last run


def _build_program():
    import concourse.bass as bass
    import concourse.mybir as mybir
    from concourse import bacc, tile
    from contextlib import ExitStack

    f32 = mybir.dt.float32
    f32r = mybir.dt.float32r
    bf16 = mybir.dt.bfloat16
    fp16 = mybir.dt.float16
    AF = mybir.ActivationFunctionType

    nc = bacc.Bacc("TRN2", target_bir_lowering=False, debug=False,
                   num_devices=N_CORES)

    # Per-core DRAM I/O (transposed activations, pre-sliced weights)
    qT = nc.dram_tensor("qT", [D_MODEL, S], mybir.dt.float16, kind="ExternalInput").ap()
    kT = nc.dram_tensor("kT", [D_MODEL, S], mybir.dt.float16, kind="ExternalInput").ap()
    vT = nc.dram_tensor("vT", [D_MODEL, S], mybir.dt.float16, kind="ExternalInput").ap()
    wq = nc.dram_tensor("wq", [D_MODEL, E], mybir.dt.float16, kind="ExternalInput").ap()
    wk = nc.dram_tensor("wk", [D_MODEL, E], mybir.dt.float16, kind="ExternalInput").ap()
    wv = nc.dram_tensor("wv", [D_MODEL, E], mybir.dt.float16, kind="ExternalInput").ap()
    wo = nc.dram_tensor("wo", [E, D_MODEL], f32r, kind="ExternalInput").ap()
    bq = nc.dram_tensor("bq", [E, 1], f32, kind="ExternalInput").ap()
    bk = nc.dram_tensor("bk", [E, 1], f32, kind="ExternalInput").ap()
    bv = nc.dram_tensor("bv", [E, 1], f32, kind="ExternalInput").ap()
    onesl = nc.dram_tensor("onesl", [1, 64], f32r, kind="ExternalInput").ap()
    onesk = nc.dram_tensor("onesk", [128, 1], mybir.dt.float16,
                           kind="ExternalInput").ap()
    zT = nc.dram_tensor("zT", [D_MODEL, S], f32, kind="ExternalOutput").ap()

    def r(ap):  # operands are natively f32r now
        return ap

    with tile.TileContext(nc) as tc, ExitStack() as ctx:
        persist = ctx.enter_context(tc.tile_pool(name="persist", bufs=1))
        const = ctx.enter_context(tc.tile_pool(name="const", bufs=1))

        # Weights resident in SBUF: [128, n_dt, E]-style views
        wq_sb = persist.tile([128, N_DT, E], fp16, tag="wq", name="wq")
        wk_sb = persist.tile([128, N_DT, E], fp16, tag="wk", name="wk")
        wv_sb = persist.tile([128, N_DT, E], fp16, tag="wv", name="wv")
        wo_sb = persist.tile([128, 2, D_MODEL], f32r, tag="wo", name="wo")
        # wk/wq first (gate the K/Q projections), split across queues;
        # wv/wo stream later behind the K inputs
        nc.sync.dma_start(wk_sb[:], wk.rearrange("(t p) e -> p t e", p=128))
        nc.scalar.dma_start(wq_sb[:], wq.rearrange("(t p) e -> p t e", p=128))
        nc.scalar.dma_start(wv_sb[:], wv.rearrange("(t p) e -> p t e", p=128))
        nc.sync.dma_start(wo_sb[:], wo.rearrange("(t p) e -> p t e", p=128))

        bq_sb = persist.tile([128, 2], f32, tag="bq", name="bq")
        bk_sb = persist.tile([128, 2], f32, tag="bk", name="bk")
        bv_sb = persist.tile([128, 2], f32, tag="bv", name="bv")
        nc.sync.dma_start(bq_sb[:], bq.rearrange("(m p) o -> p (m o)", p=128))
        nc.sync.dma_start(bk_sb[:], bk.rearrange("(m p) o -> p (m o)", p=128))
        nc.sync.dma_start(bv_sb[:], bv.rearrange("(m p) o -> p (m o)", p=128))

        from concourse.masks import make_identity
        ident = const.tile([128, 128], fp16, tag="ident", name="ident")
        make_identity(nc, ident)
        # host-provided constants: ones column (sums lhsT) and the
        # pair-broadcast selector
        ones_k = const.tile([128, 1], fp16, tag="ones_k", name="ones_k")
        ones_l = const.tile([1, 64], f32r, tag="ones_l", name="ones_l")
        nc.sync.dma_start(ones_k[:], onesk)
        nc.sync.dma_start(ones_l[:], onesl)

        # Projection outputs (transposed): pair tensors hold 2 heads each
        qh = [persist.tile([128, S], f32r, tag=f"qh{p}", name=f"qh{p}") for p in range(2)]
        kh = [persist.tile([128, S], f32r, tag=f"kh{p}", name=f"kh{p}") for p in range(2)]
        # Vh non-transposed [k, e], s-tile-major columns
        vh = persist.tile([128, N_KT * E], fp16, tag="vh", name="vh")
        # normalized attention output (transposed), pair tensors
        ot = [persist.tile([128, S], f32r, tag=f"ot{p}", name=f"ot{p}") for p in range(2)]

        stage_a = ExitStack()
        xpool = stage_a.enter_context(tc.tile_pool(name="xpool", bufs=8))
        apsum = stage_a.enter_context(
            tc.tile_pool(name="apsum", bufs=8, space="PSUM"))

        # vhT: transposed V projection [e, s] (bf16), transposed to vh after
        vhT = [persist.tile([128, S], fp16, tag=f"vhT{m}", name=f"vhT{m}")
               for m in range(2)]

        # ---- Stage A: projections (all transposed orientation) ---------
        dma_engines = (nc.sync, nc.scalar)  # two HWDGE queues
        for which, xdram, w_sb, b_sb, dst in (
            ("k", kT, wk_sb, bk_sb, kh),
            ("q", qT, wq_sb, bq_sb, qh),
            ("v", vT, wv_sb, bv_sb, vhT),
        ):
            # ps[m][n]: out rows m*128, cols n*512
            ps = [[apsum.tile([128, 512], f32, tag="aps", name="aps") for n in range(4)]
                  for m in range(2)]
            for d in range(N_DT):
                xt = xpool.tile([128, S], fp16, tag="xt", name="xt")
                dma_engines[d % 2].dma_start(xt[:], xdram[d * 128:(d + 1) * 128, :])
                for m in range(2):
                    lhsT = w_sb[:, d, m * 128:(m + 1) * 128]
                    for n in range(4):
                        nc.tensor.matmul(
                            ps[m][n][:], r(lhsT), r(xt[:, n * 512:(n + 1) * 512]),
                            start=(d == 0), stop=(d == N_DT - 1))
            for m in range(2):
                for n in range(4):
                    nc.vector.tensor_scalar_add(
                        dst[m][:, n * 512:(n + 1) * 512], ps[m][n][:],
                        b_sb[:, m:m + 1])

        # vh[s, e] = vhT^T via PE transposes (4 blocks per psum bank)
        for st in range(N_KT):
            tp = apsum.tile([128, 512], fp16, tag="aps", name="tps")                 if st % 2 == 0 else tp
            for m in range(2):
                j = (st % 2) * 2 + m
                nc.tensor.matmul(
                    tp[:, j * 128:(j + 1) * 128],
                    vhT[m][:, st * 128:(st + 1) * 128], ident[:],
                    is_transpose=True, start=True, stop=True,
                    skip_group_check=True)
                nc.vector.tensor_copy(
                    vh[:, st * E + m * 128: st * E + (m + 1) * 128],
                    tp[:, j * 128:(j + 1) * 128])

        stage_a.close()

        # ---- Stage B: attention + out_proj, per q-chunk ----------------
        scp = ctx.enter_context(tc.tile_pool(name="scp", bufs=2, space="PSUM"))
        outp = ctx.enter_context(tc.tile_pool(name="outp", bufs=2, space="PSUM"))
        sump = ctx.enter_context(tc.tile_pool(name="sump", bufs=1, space="PSUM"))
        zp = ctx.enter_context(tc.tile_pool(name="zp", bufs=1, space="PSUM"))

        ptp = ctx.enter_context(tc.tile_pool(name="ptp", bufs=8))
        rp = ctx.enter_context(tc.tile_pool(name="rp", bufs=6))
        bcp = ctx.enter_context(tc.tile_pool(name="bcp", bufs=3))
        zsb = ctx.enter_context(tc.tile_pool(name="zsb", bufs=4))

        for qc in range(N_QC):
            q0, q1 = qc * NQ, (qc + 1) * NQ
            outs = [outp.tile([128, NQ], f32, tag="outp", name="outp") for _ in range(2)]
            sums = sump.tile([128, NQ], f32, tag="sums", name="sums")

            def pv_sums(kt, pts):
                # P@V + denominator for k-tile kt (pts = pair pt tiles)
                for p in range(2):
                    for j in range(2):
                        h = 2 * p + j
                        lo, hi = j * 64, (j + 1) * 64
                        ptj = pts[p][:, j * NQ:(j + 1) * NQ]
                        # P@V (col-tiled pair: head j -> out partitions j*64)
                        nc.tensor.matmul(
                            outs[p][lo:hi, :],
                            r(vh[:, kt * E + h * 64: kt * E + (h + 1) * 64]),
                            r(ptj), start=(kt == 0), stop=(kt == N_KT - 1),
                            skip_group_check=True)
                        # softmax denominator (col-tiled 4-way, M=1)
                        nc.tensor.matmul(
                            sums[32 * h:32 * h + 1, :], r(ones_k[:]), r(ptj),
                            start=(kt == 0), stop=(kt == N_KT - 1),
                            tile_position=(0, 32 * h), skip_group_check=True)

            # k-loop software-pipelined one stage deep: scores(kt) issue on
            # PE while exp(kt-1) runs on ACT and pv/sums(kt-1) follows.
            prev_pts = None
            for kt in range(N_KT):
                k0 = kt * 128
                scs = []
                for p in range(2):
                    # both heads' scores side by side in one 2-bank psum tile
                    sc = scp.tile([128, 2 * NQ], f32, tag="sc", name="sc")
                    for j in range(2):
                        lo, hi = j * 64, (j + 1) * 64
                        nc.tensor.matmul(
                            sc[:, j * NQ:(j + 1) * NQ],
                            r(kh[p][lo:hi, k0:k0 + 128]),
                            r(qh[p][lo:hi, q0:q1]), start=True, stop=True)
                    scs.append(sc)
                if prev_pts is not None:
                    pv_sums(kt - 1, prev_pts)
                pts = []
                for p in range(2):
                    # one wide exp per pair (amortizes ACT fixed cost)
                    pt = ptp.tile([128, 2 * NQ], fp16, tag="pt", name="pt")
                    nc.scalar.activation(pt[:], scs[p][:], AF.Exp, scale=0.125)
                    pts.append(pt)
                prev_pts = pts
            pv_sums(N_KT - 1, prev_pts)
            # normalize: ot = outs * (1/sums) broadcast across partitions
            for p in range(2):
                bc_sb = bcp.tile([128, NQ], f32, tag="bc_sb", name="bc_sb")
                for j in range(2):
                    h = 2 * p + j
                    rv = rp.tile([1, NQ], f32r, tag="rv", name="rv")
                    with nc.allow_low_precision(reason="tf32 softmax recip"):
                        nc.vector.reciprocal(rv[:], sums[32 * h:32 * h + 1, :])
                    # rank-1 broadcast of 1/sum across 64 partitions (PE);
                    # separate base-0 psum tile (f32r can't col-tile)
                    bc = scp.tile([64, NQ], f32, tag="sc", name="bcps")
                    nc.tensor.matmul(bc[:], ones_l[:], rv[:],
                                     start=True, stop=True)
                    nc.vector.tensor_copy(bc_sb[j * 64:(j + 1) * 64, :], bc[:])
                nc.vector.tensor_mul(ot[p][:, q0:q1], outs[p][:], bc_sb[:])
            # out_proj partial: zT[e, q-chunk]
            for e in range(8):
                pool_, tag_ = (zp, "zps") if e % 2 == 0 else (sump, "sums")
                zps = pool_.tile([128, NQ], f32, tag=tag_, name="zps")
                for c in range(2):
                    nc.tensor.matmul(
                        zps[:], r(wo_sb[:, c, e * 128:(e + 1) * 128]),
                        r(ot[c][:, q0:q1]), start=(c == 0), stop=(c == 1))
                zt_sb = zsb.tile([128, NQ], f32, tag="zt_sb", name="zt_sb")
                nc.vector.tensor_copy(zt_sb[:], zps[:])
                dma_engines[e % 2].dma_start(
                    zT[e * 128:(e + 1) * 128, q0:q1], zt_sb[:])

    nc.compile()
    return nc


def _get_program():
    global _PROGRAM
    if _PROGRAM is None:
        _PROGRAM = _build_program()
    return _PROGRAM


ONESL_NP = None
ONESK_NP = None


def _init_consts():
    global ONESL_NP, ONESK_NP
    if ONESL_NP is None:
        import ml_dtypes
        ONESL_NP = np.ones((1, 64), dtype=np.float32)
        ONESK_NP = np.ones((128, 1), np.float16)


def _make_in_maps(q, k, v, Wq, bq, Wk, bk, Wv, bv, Wo):
    _init_consts()
    f32 = np.float32
    xT = {}
    for b in range(B):
        xT[("q", b)] = np.ascontiguousarray(q[b].T, dtype=np.float16)
        xT[("k", b)] = np.ascontiguousarray(k[b].T, dtype=np.float16)
        xT[("v", b)] = np.ascontiguousarray(v[b].T, dtype=np.float16)
    wslices = {}
    for g in range(4):
        sl = slice(g * E, (g + 1) * E)
        wslices[("wq", g)] = np.ascontiguousarray(Wq[sl, :].T, dtype=np.float16)
        wslices[("wk", g)] = np.ascontiguousarray(Wk[sl, :].T, dtype=np.float16)
        wslices[("wv", g)] = np.ascontiguousarray(Wv[sl, :].T, dtype=np.float16)
        wslices[("wo", g)] = np.ascontiguousarray(Wo[:, sl].T, dtype=f32)
        wslices[("bq", g)] = np.ascontiguousarray(bq[sl].reshape(E, 1), dtype=f32)
        wslices[("bk", g)] = np.ascontiguousarray(bk[sl].reshape(E, 1), dtype=f32)
        wslices[("bv", g)] = np.ascontiguousarray(bv[sl].reshape(E, 1),
                                                   dtype=f32)
    in_maps = []
    for c in range(N_CORES):
        b, g = c // 4, c % 4
        in_maps.append({
            "onesl": ONESL_NP, "onesk": ONESK_NP,
            "qT": xT[("q", b)], "kT": xT[("k", b)], "vT": xT[("v", b)],
            "wq": wslices[("wq", g)], "wk": wslices[("wk", g)],
            "wv": wslices[("wv", g)], "wo": wslices[("wo", g)],
            "bq": wslices[("bq", g)], "bk": wslices[("bk", g)],
            "bv": wslices[("bv", g)],
        })
    return in_maps


def _numpy_fallback(q, k, v, mask, Wq, bq, Wk, bk, Wv, bv, Wo, bo):
    # Only used if mask is not all-True (never the case for this problem).
    def proj(x, W, b_):
        y = x @ W.T + b_
        return y.reshape(B, S, NUM_HEADS, DK).transpose(0, 2, 1, 3)
    qh, kh, vh = proj(q, Wq, bq), proj(k, Wk, bk), proj(v, Wv, bv)
    sc = np.einsum("bhqd,bhkd->bhqk", qh, kh) / np.sqrt(DK)
    sc = np.where(mask, sc, np.float32(-1e9))
    sc = sc - sc.max(-1, keepdims=True)
    p = np.exp(sc)
    p /= p.sum(-1, keepdims=True)
    o = np.einsum("bhqk,bhkd->bhqd", p, vh)
    o = o.transpose(0, 2, 1, 3).reshape(B, S, D_MODEL)
    return (o @ Wo.T + bo).astype(np.float32)


def kernel(q, k, v, mask, Wq, bq, Wk, bk, Wv, bv, Wo, bo):
    q = np.asarray(q, dtype=np.float32)
    k = np.asarray(k, dtype=np.float32)
    v = np.asarray(v, dtype=np.float32)
    Wq, Wk, Wv, Wo = (np.asarray(w, dtype=np.float32) for w in (Wq, Wk, Wv, Wo))
    bq, bk, bv, bo = (np.asarray(x, dtype=np.float32) for x in (bq, bk, bv, bo))
    if not np.all(np.asarray(mask)):
        return _numpy_fallback(q, k, v, np.asarray(mask), Wq, bq, Wk, bk,
                               Wv, bv, Wo, bo)

    from concourse.bass_utils import run_bass_kernel_spmd
    nc = _get_program()
    in_maps = _make_in_maps(q, k, v, Wq, bq, Wk, bk, Wv, bv, Wo)
    res = run_bass_kernel_spmd(nc, in_maps, core_ids=list(range(N_CORES)),
                               **_RUN_KWARGS)
    global _LAST_RESULTS
    _LAST_RESULTS = res
    out = np.empty((B, S, D_MODEL), dtype=np.float32)
    for b in range(B):
        acc = res.results[4 * b]["zT"].astype(np.float32).copy()
        for g in range(1, 4):
            acc += res.results[4 * b + g]["zT"]
        out[b] = acc.T + bo
    return out



# revision 36
# speedup vs baseline: 1.4603x; 1.4603x over previous
"""Trainium2 Bass kernel for nn_MultiHeadAttention (B=2, S=2048, d_model=1024, H=16).

Sharding (8 cores): data-parallel over B (2) x tensor-parallel over head groups
(4 groups of 4 heads).  Each core computes its head-group's Q/K/V projections
(column-sharded weights), attention for its 4 heads, and a row-parallel
out_proj partial product.  The host sums the 4 partials per batch (the
"all-reduce") and adds the output bias.

Cost-model-driven design (matmul time = out-free-dim cycles, independent of
M/K; Ldweights engine-free):
  - scores computed transposed S_T[k, q] = Kh^T@Qh (kh stationary), exp'd on
    ACT in [128, 1024] calls (4 heads x 256-q chunk per k-tile),
  - P@V uses the P tile as the STATIONARY operand and V as moving, with a
    ones-column appended to V: out[q, 65] per head -- column 64 is the
    softmax denominator for free (no separate sums matmuls, no reciprocal
    broadcast matmuls),
  - normalization = per-partition DVE tensor_scalar multiply (q on
    partitions), then PE-transpose back to [e, q] for the out_proj,
  - V projection computed directly non-transposed (vT tile stationary, Wv
    moving) -- no V transposes,
  - all DMAs on the SP queue (ACT queue kept exp-only), out_proj partials
    copied psum->SBUF on the idle Pool engine, DMA'd per chunk-pair,
  - fp16 operands everywhere (fp32 PSUM accumulation).
"""

import sys
import numpy as np

for _p in ("/opt/trn_rl_repo", "/root/.axon_site/_ro/trn_rl_repo"):
    if _p not in sys.path:
        sys.path.append(_p)

D_MODEL = 1024
NUM_HEADS = 16
DK = 64
B = 2
S = 2048
N_CORES = 8
HPC = 4               # heads per core
E = HPC * DK          # 256 features per core
NQ = 256              # q-chunk size
N_QC = S // NQ        # 8 q chunks
N_KT = S // 128       # 16 k tiles
N_DT = D_MODEL // 128  # 8 contraction tiles for projections
EV = DK + 1           # V feature block incl. ones column (denominator)

_PROGRAM = None
_RUN_KWARGS = {}      # test harness may set {"trace": True}
_LAST_RESULTS = None  # BassKernelResults of the last run


def _build_program():
    import concourse.bass as bass
    import concourse.mybir as mybir
    from concourse import bacc, tile
    from contextlib import ExitStack

    f32 = mybir.dt.float32
    fp16 = mybir.dt.float16
    i16 = mybir.dt.int16
    AF = mybir.ActivationFunctionType
    ALU = mybir.AluOpType

    # exp split: for 4 of 16 k-tiles the DVE computes a Schraudolph-style
    # bit-trick exp in ONE op (i16 = round(a*s + b), read back by the P@V
    # matmul through a bitcast-to-fp16 AP).  The bit pattern evaluates
    # C1*exp(s/8) with C1 = 1.04085 (mean mantissa-interpolation factor,
    # +-3% deviation); the ACT k-tiles match the constant via the free
    # activation bias so softmax cancels it row-wise.
    EXP_BIAS = float(np.log(1.0408461))
    SCHRA_A = 0.125 * 1024.0 / float(np.log(2.0))
    SCHRA_B = 15.0 * 1024.0
    DVE_KTS = (1, 3, 5, 7, 9, 11, 13, 15)

    nc = bacc.Bacc("TRN2", target_bir_lowering=False, debug=False,
                   num_devices=N_CORES)

    # Per-core DRAM I/O (transposed activations, pre-sliced weights)
    qT = nc.dram_tensor("qT", [D_MODEL, S], fp16, kind="ExternalInput").ap()
    kT = nc.dram_tensor("kT", [D_MODEL, S], fp16, kind="ExternalInput").ap()
    vT = nc.dram_tensor("vT", [D_MODEL, S], fp16, kind="ExternalInput").ap()
    wq = nc.dram_tensor("wq", [D_MODEL, E], fp16, kind="ExternalInput").ap()
    wk = nc.dram_tensor("wk", [D_MODEL, E], fp16, kind="ExternalInput").ap()
    wv = nc.dram_tensor("wv", [D_MODEL, E], fp16, kind="ExternalInput").ap()
    wo = nc.dram_tensor("wo", [E, D_MODEL], fp16, kind="ExternalInput").ap()
    bq = nc.dram_tensor("bq", [E, 1], f32, kind="ExternalInput").ap()
    bk = nc.dram_tensor("bk", [E, 1], f32, kind="ExternalInput").ap()
    bvb = nc.dram_tensor("bvb", [128, E], f32, kind="ExternalInput").ap()
    zT = nc.dram_tensor("zT", [D_MODEL, S], fp16, kind="ExternalOutput").ap()

    with tile.TileContext(nc) as tc, ExitStack() as ctx:
        persist = ctx.enter_context(tc.tile_pool(name="persist", bufs=1))
        const = ctx.enter_context(tc.tile_pool(name="const", bufs=1))

        # ---- weights + biases resident in SBUF --------------------------
        wq_sb = persist.tile([128, N_DT, E], fp16, tag="wq", name="wq")
        wk_sb = persist.tile([128, N_DT, E], fp16, tag="wk", name="wk")
        wv_sb = persist.tile([128, N_DT, E], fp16, tag="wv", name="wv")
        wo_sb = persist.tile([128, 2, D_MODEL], fp16, tag="wo", name="wo")
        bq_sb = persist.tile([128, 2], f32, tag="bq", name="bq")
        bk_sb = persist.tile([128, 2], f32, tag="bk", name="bk")
        bvb_sb = persist.tile([128, E], f32, tag="bvb", name="bvb")

        wkv = wk.rearrange("(t p) e -> p t e", p=128)
        nc.sync.dma_start(wk_sb[:, 0:4, :], wkv[:, 0:4, :])
        nc.sync.dma_start(wk_sb[:, 4:8, :], wkv[:, 4:8, :])

        # input stream tiles: [128, 1024] halves of each d-tile
        xpool = ctx.enter_context(tc.tile_pool(name="xpool", bufs=16))

        def load_half(src, d, h):
            xt = xpool.tile([128, S // 2], fp16, tag="xt", name="xt")
            nc.sync.dma_start(xt[:], src[d * 128:(d + 1) * 128,
                                         h * (S // 2):(h + 1) * (S // 2)])
            return xt

        # k first (gates everything), then q first-half, then k half1,
        # then v (consumed from chunk0-kt8 on), then q half1 (chunk 4+)
        kts = [[None, None] for _ in range(N_DT)]
        qts = [[None, None] for _ in range(N_DT)]
        vts = [[None, None] for _ in range(N_DT)]
        for d in range(N_DT):
            kts[d][0] = load_half(kT, d, 0)
        nc.sync.dma_start(wq_sb[:], wq.rearrange("(t p) e -> p t e", p=128))
        nc.sync.dma_start(bk_sb[:], bk.rearrange("(m p) o -> p (m o)", p=128))
        nc.sync.dma_start(bq_sb[:], bq.rearrange("(m p) o -> p (m o)", p=128))
        for d in range(N_DT):
            qts[d][0] = load_half(qT, d, 0)
        for d in range(N_DT):
            kts[d][1] = load_half(kT, d, 1)
        nc.sync.dma_start(wv_sb[:], wv.rearrange("(t p) e -> p t e", p=128))
        nc.sync.dma_start(bvb_sb[:], bvb)
        for d in range(N_DT):
            vts[d][0] = load_half(vT, d, 0)
        for d in range(N_DT):
            vts[d][1] = load_half(vT, d, 1)
        for d in range(N_DT):
            qts[d][1] = load_half(qT, d, 1)
        nc.sync.dma_start(wo_sb[:], wo.rearrange("(t p) e -> p t e", p=128))

        from concourse.masks import make_identity
        ident = const.tile([128, 128], fp16, tag="ident", name="ident")
        make_identity(nc, ident)
        ebias = const.tile([128, 1], f32, tag="ebias", name="ebias")
        nc.gpsimd.memset(ebias[:], EXP_BIAS)

        # ---- persistent activations ------------------------------------
        qh = [persist.tile([128, S], fp16, tag=f"qh{p}", name=f"qh{p}")
              for p in range(2)]
        kh = [persist.tile([128, S], fp16, tag=f"kh{p}", name=f"kh{p}")
              for p in range(2)]
        # V projection non-transposed: [s(128-tile), kt, head, 65]
        vh65 = persist.tile([128, N_KT * HPC * EV], fp16, tag="vh65",
                            name="vh65")
        vh65v = vh65.rearrange("p (t h c) -> p t h c", t=N_KT, h=HPC)
        # ones columns for the softmax denominators
        nc.gpsimd.memset(vh65v[:, :, :, DK:DK + 1], 1.0)

        # ---- PSUM pools (exactly 8 banks) ------------------------------
        # HW rule: one (non-transpose) matmul output region per PSUM bank
        # (partition-splits and transposes may share; column-splits may not).
        scorep = ctx.enter_context(
            tc.tile_pool(name="scorep", bufs=4, space="PSUM"))  # 4x1 bank
        pvp = ctx.enter_context(
            tc.tile_pool(name="pvp", bufs=2, space="PSUM"))     # 2x1 bank
        miscp = ctx.enter_context(
            tc.tile_pool(name="miscp", bufs=2, space="PSUM"))   # 2x1 bank

        ptp = ctx.enter_context(tc.tile_pool(name="ptp", bufs=40))
        otp = ctx.enter_context(tc.tile_pool(name="otp", bufs=4))
        otTp = ctx.enter_context(tc.tile_pool(name="otTp", bufs=4))
        rcpp = ctx.enter_context(tc.tile_pool(name="rcpp", bufs=6))
        zsbp = ctx.enter_context(tc.tile_pool(name="zsbp", bufs=2))

        # ---- projection helpers ----------------------------------------
        def proj_pair(xhalves, w_sb, b_sb, dst, nb):
            # both m accumulation groups of a seq block, d-outer so the
            # d-step stream chases the input DMA arrivals
            xh, off = nb // 2, (nb % 2) * 512
            ps = [miscp.tile([128, 512], f32, tag="misc", name="pps")
                  for _ in range(2)]
            for d in range(N_DT):
                for m in range(2):
                    nc.tensor.matmul(
                        ps[m][:], w_sb[:, d, m * 128:(m + 1) * 128],
                        xhalves[d][xh][:, off:off + 512],
                        start=(d == 0), stop=(d == N_DT - 1))
            for m in range(2):
                nc.vector.tensor_scalar_add(
                    dst[m][:, nb * 512:(nb + 1) * 512], ps[m][:],
                    b_sb[:, m:m + 1])

        def proj_v_st(st):
            # V projection, direct: out [s(128), e(256)] for s-tile st
            xh = st // 8
            ps = miscp.tile([128, 512], f32, tag="misc", name="vps")
            for d in range(N_DT):
                nc.tensor.matmul(
                    ps[:, 0:E],
                    vts[d][xh][:, (st % 8) * 128:(st % 8) * 128 + 128],
                    wv_sb[:, d, :], start=(d == 0), stop=(d == N_DT - 1))
            nc.vector.tensor_tensor(
                vh65v[:, st, :, 0:DK],
                ps[:, 0:E].rearrange("p (h j) -> p h j", h=HPC),
                bvb_sb.rearrange("p (h j) -> p h j", h=HPC),
                op=mybir.AluOpType.add)

        # ---- per-slot emission pieces ----------------------------------
        zsb = {}
        pts_of = {}   # chunk -> list of pt tiles (one per kt)
        pv_of = {}    # (chunk, sweep) -> pv psum tile [128, 65]
        ot_of = {}    # (chunk, qb) -> normalized ot_sb tile
        otT_of = {}   # chunk-pair -> [otT bb0, bb1] (two chunks wide)

        def emit_scores_exp_kt(c, kt):
            # one [128, 256] score tile (own PSUM bank) per head; exp per
            # (kt, head): ACT exact when (kt+h) even, DVE bit-trick when
            # odd -- except two designated kts per 8 where ACT also takes
            # the first DVE quarter (engine load balance; also lowers the
            # approximated fraction slightly).
            pt = ptp.tile([128, HPC * NQ], fp16, tag="pt", name="pt")
            for h in range(HPC):
                sc = scorep.tile([128, NQ], f32, tag="sc", name="sc")
                p, j = h // 2, h % 2
                nc.tensor.matmul(
                    sc[:],
                    kh[p][j * 64:(j + 1) * 64, kt * 128:(kt + 1) * 128],
                    qh[p][j * 64:(j + 1) * 64, c * NQ:(c + 1) * NQ],
                    start=True, stop=True, skip_group_check=True)
                dst = pt[:, h * NQ:(h + 1) * NQ]
                if (kt + h) % 2 == 1:
                    nc.vector.tensor_scalar(
                        dst.bitcast(i16), sc[:], SCHRA_A, SCHRA_B,
                        op0=ALU.mult, op1=ALU.add)
                else:
                    nc.scalar.activation(dst, sc[:], AF.Exp, scale=0.125,
                                         bias=ebias[:])
            pts_of.setdefault(c, []).append(pt)

        def emit_pv_half(c, s, half):
            # P@V sweep for (qb = s//4, head = s%4): 8 accumulating
            # matmuls (k-tiles half*8..half*8+7) into one [128, 65] bank
            qb, h = s // 4, s % 4
            if half == 0:
                pv_of[(c, s)] = pvp.tile([128, EV], f32, tag="pv", name="pv")
            pv = pv_of[(c, s)]
            pts = pts_of[c]
            for kt in range(half * 8, half * 8 + 8):
                nc.tensor.matmul(
                    pv[:],
                    pts[kt][:, h * NQ + qb * 128:h * NQ + qb * 128 + 128],
                    vh65v[:, kt, h, :],
                    start=(kt == 0), stop=(kt == N_KT - 1),
                    skip_group_check=True)

        def emit_norm_s(c, s):
            # recip of the denominator column + normalized copy to SBUF
            qb, h = s // 4, s % 4
            pv = pv_of.pop((c, s))
            rc = rcpp.tile([128, 1], f32, tag="rc", name="rc")
            nc.vector.reciprocal(rc[:], pv[:, DK:DK + 1])
            if h == 0:
                ot_of[(c, qb)] = otp.tile([128, E], fp16, tag="ot",
                                          name="ot")
            if h % 2 == 0:
                nc.scalar.activation(
                    ot_of[(c, qb)][:, h * DK:(h + 1) * DK], pv[:, 0:DK],
                    AF.Copy, scale=rc[:])
            else:
                nc.vector.tensor_scalar_mul(
                    ot_of[(c, qb)][:, h * DK:(h + 1) * DK], pv[:, 0:DK],
                    rc[:])

        def emit_transpose_qb(c, qb):
            # [q, e] -> [e, q] via PE transposes (transposes may share a
            # psum bank); otT tiles span a chunk PAIR (512 q columns)
            cp = c // 2
            if cp not in otT_of:
                otT_of[cp] = [otTp.tile([128, 2 * NQ], fp16, tag="otT",
                                        name="otT") for _ in range(2)]
            ot = ot_of.pop((c, qb))
            tp = miscp.tile([128, 256], fp16, tag="misc", name="tp")
            for bb in range(2):
                nc.tensor.matmul(
                    tp[:, bb * 128:(bb + 1) * 128],
                    ot[:, bb * 128:(bb + 1) * 128], ident[:],
                    is_transpose=True, start=True, stop=True,
                    skip_group_check=True)
                nc.vector.tensor_copy(
                    otT_of[cp][bb][:, (c % 2) * NQ + qb * 128:
                                   (c % 2) * NQ + qb * 128 + 128],
                    tp[:, bb * 128:(bb + 1) * 128])
            if qb == 1:
                pts_of.pop(c, None)

        def emit_outproj(cp, eo):
            # out_proj partial for chunk-pair cp (512 q): zT[eo-block, q]
            otT = otT_of[cp]
            zps = miscp.tile([128, 512], f32, tag="misc", name="zps")
            for cc in range(2):
                nc.tensor.matmul(
                    zps[:], wo_sb[:, cc, eo * 128:(eo + 1) * 128],
                    otT[cc][:], start=(cc == 0), stop=(cc == 1),
                    skip_group_check=True)
            if eo == 0:
                zsb[cp] = zsbp.tile([128, 8, 2 * NQ], fp16, tag="zsb",
                                    name="zs")
            if eo % 2 == 0:
                nc.scalar.activation(zsb[cp][:, eo, :], zps[:], AF.Copy)
            else:
                nc.vector.tensor_copy(zsb[cp][:, eo, :], zps[:])
            if eo == 7:
                nc.sync.dma_start(
                    zT.rearrange("(eo p) s -> p eo s", p=128)[
                        :, :, cp * 2 * NQ:(cp + 1) * 2 * NQ], zsb[cp][:])

        # ---- software pipeline -----------------------------------------
        # lead-in: k half0 projection + first q block
        proj_pair(kts, wk_sb, bk_sb, kh, 0)
        proj_pair(qts, wq_sb, bq_sb, qh, 0)
        proj_pair(kts, wk_sb, bk_sb, kh, 1)

        extras = {}

        def add_extra(cs, kt, fn):
            extras.setdefault((cs, kt), []).append(fn)

        # chunk0: k half1 at kt4/6 (needed by scores kt8+),
        #         V st0-7 at kt8-15, V st8-15 at step1 kt0-7 (needed by
        #         the PV(c0) sweeps which start at step1 slot 8)
        add_extra(0, 4, lambda: proj_pair(kts, wk_sb, bk_sb, kh, 2))
        add_extra(0, 6, lambda: proj_pair(kts, wk_sb, bk_sb, kh, 3))
        for i in range(8):
            add_extra(0, 8 + i, lambda st=i: proj_v_st(st))
        for i in range(8):
            add_extra(1, i, lambda st=8 + i: proj_v_st(st))
        add_extra(1, 8, lambda: proj_pair(qts, wq_sb, bq_sb, qh, 1))
        add_extra(2, 5, lambda: proj_pair(qts, wq_sb, bq_sb, qh, 2))
        add_extra(3, 5, lambda: proj_pair(qts, wq_sb, bq_sb, qh, 3))

        # Schedule per step/slot.  Sweeps of chunk c: qb0 (s 0-3) at step
        # c+1 slots 8-15, qb1 (s 4-7) at step c+2 slots 0-7; norms trail
        # each sweep; transposes at step c+2 slots 1/9; out_proj per
        # chunk-pair at the odd chunk's step+2, slots 11-14.
        for step in range(N_QC + 2):
            for kt in range(N_KT):
                # qb1 sweeps + trailing norms for chunk step-2
                c2 = step - 2
                if 0 <= c2 <= N_QC - 1:
                    if kt < 8:
                        emit_pv_half(c2, 4 + kt // 2, kt % 2)
                    if kt == 0:
                        emit_norm_s(c2, 3)
                    if kt in (2, 4, 6, 8) and kt // 2 + 3 <= 7:
                        emit_norm_s(c2, kt // 2 + 3)
                    if kt == 1:
                        emit_transpose_qb(c2, 0)
                    if kt == 9:
                        emit_transpose_qb(c2, 1)
                    if c2 % 2 == 1 and 11 <= kt <= 14:
                        emit_outproj(c2 // 2, 2 * (kt - 11))
                        emit_outproj(c2 // 2, 2 * (kt - 11) + 1)
                # qb0 sweeps + norms for chunk step-1
                c1 = step - 1
                if 0 <= c1 <= N_QC - 1 and kt >= 8:
                    emit_pv_half(c1, (kt - 8) // 2, kt % 2)
                    if kt in (10, 12, 14):
                        emit_norm_s(c1, (kt - 10) // 2)
                for fn in extras.get((step, kt), ()):
                    fn()
                if step <= N_QC - 1:
                    emit_scores_exp_kt(step, kt)

    nc.compile()
    return nc


def _get_program():
    global _PROGRAM
    if _PROGRAM is None:
        _PROGRAM = _build_program()
    return _PROGRAM


def _make_in_maps(q, k, v, Wq, bq, Wk, bk, Wv, bv, Wo):
    f32 = np.float32
    xT = {}
    for b in range(B):
        xT[("q", b)] = np.ascontiguousarray(q[b].T, dtype=np.float16)
        xT[("k", b)] = np.ascontiguousarray(k[b].T, dtype=np.float16)
        xT[("v", b)] = np.ascontiguousarray(v[b].T, dtype=np.float16)
    wslices = {}
    for g in range(4):
        sl = slice(g * E, (g + 1) * E)
        wslices[("wq", g)] = np.ascontiguousarray(Wq[sl, :].T, dtype=np.float16)
        wslices[("wk", g)] = np.ascontiguousarray(Wk[sl, :].T, dtype=np.float16)
        wslices[("wv", g)] = np.ascontiguousarray(Wv[sl, :].T, dtype=np.float16)
        wslices[("wo", g)] = np.ascontiguousarray(Wo[:, sl].T, dtype=np.float16)
        wslices[("bq", g)] = np.ascontiguousarray(bq[sl].reshape(E, 1), dtype=f32)
        wslices[("bk", g)] = np.ascontiguousarray(bk[sl].reshape(E, 1), dtype=f32)
        wslices[("bvb", g)] = np.ascontiguousarray(
            np.tile(bv[sl].reshape(1, E), (128, 1)), dtype=f32)
    in_maps = []
    for c in range(N_CORES):
        b, g = c // 4, c % 4
        in_maps.append({
            "qT": xT[("q", b)], "kT": xT[("k", b)], "vT": xT[("v", b)],
            "wq": wslices[("wq", g)], "wk": wslices[("wk", g)],
            "wv": wslices[("wv", g)], "wo": wslices[("wo", g)],
            "bq": wslices[("bq", g)], "bk": wslices[("bk", g)],
            "bvb": wslices[("bvb", g)],
        })
    return in_maps


def _numpy_fallback(q, k, v, mask, Wq, bq, Wk, bk, Wv, bv, Wo, bo):
    # Only used if mask is not all-True (never the case for this problem).
    def proj(x, W, b_):
        y = x @ W.T + b_
        return y.reshape(B, S, NUM_HEADS, DK).transpose(0, 2, 1, 3)
    qh, kh, vh = proj(q, Wq, bq), proj(k, Wk, bk), proj(v, Wv, bv)
    sc = np.einsum("bhqd,bhkd->bhqk", qh, kh) / np.sqrt(DK)
    sc = np.where(mask, sc, np.float32(-1e9))
    sc = sc - sc.max(-1, keepdims=True)
    p = np.exp(sc)
    p /= p.sum(-1, keepdims=True)
    o = np.einsum("bhqk,bhkd->bhqd", p, vh)
    o = o.transpose(0, 2, 1, 3).reshape(B, S, D_MODEL)
    return (o @ Wo.T + bo).astype(np.float32)


def kernel(q, k, v, mask, Wq, bq, Wk, bk, Wv, bv, Wo, bo):
    q = np.asarray(q, dtype=np.float32)
    k = np.asarray(k, dtype=np.float32)
    v = np.asarray(v, dtype=np.float32)
    Wq, Wk, Wv, Wo = (np.asarray(w, dtype=np.float32) for w in (Wq, Wk, Wv, Wo))
    bq, bk, bv, bo = (np.asarray(x, dtype=np.float32) for x in (bq, bk, bv, bo))
    if not np.all(np.asarray(mask)):
        return _numpy_fallback(q, k, v, np.asarray(mask), Wq, bq, Wk, bk,
                               Wv, bv, Wo, bo)

    from concourse.bass_utils import run_bass_kernel_spmd
    nc = _get_program()
    in_maps = _make_in_maps(q, k, v, Wq, bq, Wk, bk, Wv, bv, Wo)
    res = run_bass_kernel_spmd(nc, in_maps, core_ids=list(range(N_CORES)),
                               **_RUN_KWARGS)
    global _LAST_RESULTS
    _LAST_RESULTS = res
    out = np.empty((B, S, D_MODEL), dtype=np.float32)
    for b in range(B):
        acc = res.results[4 * b]["zT"].astype(np.float32)
        for g in range(1, 4):
            acc = acc + res.results[4 * b + g]["zT"].astype(np.float32)
        out[b] = acc.T + bo
    return out
